# revision 12
# baseline (speedup 1.0000x reference)
"""CTC mean loss on 8 trn2 NeuronCores (Bass/Tile) — restructured.

Per example: linear-domain CTC forward DP with per-t normalizer
g = pb + den/C. Label-position recurrences run as tensor_tensor_scan
over time; 4 label chunks x 4 time segments wavefront over 128
partitions, with per-wave boundary rescaling (log-scale tracked in Scol).

Layout (vs the previous version): CURL[., j, 0] holds the b-boundary so
the CB scan runs with an immediate 0.0 initial (AP initials cost ~86ns
extra per scan); CURL[., j, 1+t] = l_j[t].  Stats for segment k and the
emit (gathered-logit exp) for wave w+1 are issued behind jloop(w) and
run on ACT/DMA/gpsimd under the DVE wave grind.  The rescale chain uses
ACT exp instead of exact exp2 bitcast round-trips (S absorbs the tiny
log error), and outputs are staged into contiguous tiles and shipped
once at the end.
"""
import numpy as np

# problem constants (fixed by the spec)
B, T, C, L = 256, 512, 128, 128
NCORE = 8
BLOC = B // NCORE          # 32 examples per core
NCH, JW = 4, 32            # label chunks x positions per chunk (i = 32q+j+1)
NSEG, TSEG = 4, 128        # time segments
NW = NSEG + NCH - 1        # 7 waves
BLANK = C - 1
LT = 38.0                  # log rescale target
TINY = 4e-18
KLN = float(np.log(2.0) / (1 << 23))
FB = float(127 << 23)

_PROG = {}


def _wave_ranges(w):
    qlo, qhi = max(0, w - (NSEG - 1)), min(NCH - 1, w)
    return qlo, qhi, 32 * qlo, 32 * (qhi + 1)


MAXEND = {0: 128, 32: 64, 64: 128, 96: 128}


def qsplit(a, b):
    """Decompose [a, b) into SBUF-legal partition ranges (quadrant rules)."""
    out = []
    while a < b:
        e = min(b, MAXEND[a])
        if e <= a:
            e = min(c for c in (32, 64, 96, 128) if c > a)
        out.append((a, e))
        a = e
    return out


def _patch_drain():
    """This container's walrus rejects TPB_CTRL drains with >1 sem wait;
    split the TileContext exit drain into one-wait-per-drain instructions."""
    import bass_rust
    import concourse.tile as tile_mod
    from concourse.tile import ScopedClock

    if getattr(tile_mod.TileContext, "_drain_split_patch", False):
        return

    def patched(self, tick_clock, wait_clock):
        drain_inst = self.nc.sync.drain()
        wait_clock.add_sem_waits(
            drain_inst.ins, ScopedClock({None: tick_clock.global_clock})
        )
        si = drain_inst.ins.sync_info
        waits = list(si.on_wait) if si is not None else []
        if len(waits) > 1:
            drain_inst.ins.sync_info = bass_rust.SyncInfo(
                on_wait=waits[:1], on_update=list(si.on_update)
            )
            for i in range(1, len(waits)):
                extra = self.nc.sync.drain()
                extra.ins.sync_info = bass_rust.SyncInfo(
                    on_wait=[waits[i]], on_update=[]
                )
        self.nc.all_engine_barrier()
        popped = self.nc._tile_sem_poison_stack.pop()
        assert popped is self._sem_poison
        self.nc.clear_and_free_semaphores(list(self.sems.allocated().values()))
        self.nc.all_engine_barrier()

    tile_mod.TileContext._drain_and_barrier = patched
    tile_mod.TileContext._drain_split_patch = True


def _split_waits(nc):
    """This container's walrus accepts at most ONE sem wait per instruction;
    hoist extra waits onto same-engine NoOps inserted just before."""
    import bass_rust

    cnt = 0
    for f in nc.m.functions:
        for bb in f.blocks:
            new = []
            changed = False
            for inst in bb.instructions:
                si = inst.sync_info
                waits = list(si.on_wait) if si is not None else []
                if len(waits) > 1:
                    changed = True
                    for wt in waits[:-1]:
                        cnt += 1
                        nop = bass_rust.InstNoOp(
                            name=f"I-wsplit-{cnt}", engine=inst.engine
                        )
                        nop.sync_info = bass_rust.SyncInfo(on_wait=[wt], on_update=[])
                        new.append(nop)
                    inst.sync_info = bass_rust.SyncInfo(
                        on_wait=[waits[-1]], on_update=list(si.on_update)
                    )
                new.append(inst)
            if changed:
                bb.instructions = new
    return cnt


def build_program(split_waits=True):
    import concourse.bass as bass
    import concourse.mybir as mybir
    from concourse.tile import TileContext
    from concourse.masks import make_identity

    _patch_drain()
    f32 = mybir.dt.float32
    bf16 = mybir.dt.bfloat16
    i32 = mybir.dt.int32
    Alu = mybir.AluOpType
    Act = mybir.ActivationFunctionType
    Ax = mybir.AxisListType

    nc = bass.Bass()
    shiftm = nc.declare_dram_parameter("shiftm", [128, 256], f32, isOutput=False)
    zlw = nc.declare_dram_parameter("zlw", [512 * JW, TSEG], bf16, isOutput=False)
    ydN = nc.declare_dram_parameter("ydN", [BLOC * NSEG * TSEG, C], f32, isOutput=False)
    zbk = nc.declare_dram_parameter("zbk", [BLOC, NSEG * TSEG], f32, isOutput=False)
    dcol_in = nc.declare_dram_parameter("dcol", [128, JW], f32, isOutput=False)
    # segment-0 stats precomputed host-side (waves 0..3 need them at start;
    # segments 1-3 are computed on-device, overlapped under the DP waves)
    qb0_in = nc.declare_dram_parameter("qb0", [BLOC, TSEG], bf16, isOutput=False)
    st0_in = nc.declare_dram_parameter("st0", [BLOC, 3], f32, isOutput=False)
    out_st = nc.declare_dram_parameter("out_st", [128, 2 * JW + 1], f32,
                                       isOutput=True)
    out_ll = nc.declare_dram_parameter("out_ll", [BLOC, 2 * NSEG], f32,
                                       isOutput=True)

    TP2 = TSEG + 2   # CURL free width per j: [bbnd, l_0 .. l_TSEG]
    TP1 = TSEG + 1

    with TileContext(nc) as tc:
        with (
            tc.tile_pool(name="pers", bufs=1) as pers,
            tc.tile_pool(name="psum", bufs=1, space="PSUM") as psum,
            tc.tile_pool(name="load", bufs=6) as loadp,
            tc.tile_pool(name="work", bufs=6) as workp,
            tc.tile_pool(name="cols", bufs=2) as colp,
        ):
            # ---------------- persistent state ----------------
            CURL = [pers.tile([128, JW, TP2], bf16, name=f"curl{p}", tag=f"curl{p}")
                    for p in range(2)]
            CURB = [pers.tile([128, JW, TP1], bf16, name=f"curb{p}", tag=f"curb{p}")
                    for p in range(2)]
            QN = [pers.tile([128, JW, TP1], bf16, name=f"qn{p}", tag=f"qn{p}")
                  for p in range(3)]
            UT = pers.tile([128, JW, TP1], bf16, name="ut", tag="ut")
            QBX = [pers.tile([128, TP1], bf16, name=f"qbx{w}", tag=f"qbx{w}")
                   for w in range(NW)]
            EB = [pers.tile([128, 1], f32, name=f"eb{w}", tag=f"eb{w}")
                  for w in range(NW)]
            PREVB = [pers.tile([128, TSEG], bf16, name=f"prevb{p}", tag=f"prevb{p}")
                     for p in range(2)]
            B0 = [pers.tile([32, TP1], f32, name=f"b0{p}", tag=f"b0{p}")
                  for p in range(2)]
            B0u = pers.tile([32, TP1], f32, name="b0u", tag="b0u")
            Z33 = pers.tile([32, TP1], f32, name="z33", tag="z33")
            DCOL = pers.tile([128, JW], f32, name="dcol", tag="dcol")
            IDENT = pers.tile([128, 128], f32, name="ident", tag="ident")
            SHI = pers.tile([128, 128], f32, name="shi", tag="shi")
            SHFF = pers.tile([128, 128], f32, name="shff", tag="shff")
            SHF = pers.tile([128, 128], bf16, name="shf", tag="shf")
            SDEN = pers.tile([128, NSEG * BLOC], f32, name="sden", tag="sden")
            ZBK = pers.tile([BLOC, NSEG * TSEG], f32, name="zbkt", tag="zbkt")
            LL = pers.tile([BLOC, 2 * NSEG], f32, name="ll", tag="ll")
            STOUT = pers.tile([128, 2 * JW + 1], f32, name="stout", tag="stout")
            Scol = pers.tile([128, 1], f32, name="scol", tag="scol")
            rcol = pers.tile([128, 1], f32, name="rcol", tag="rcol")
            ARGS = pers.tile([128, 2], f32, name="args", tag="args")
            RCE = pers.tile([128, 2], f32, name="rce", tag="rce")

            zlw_v = zlw.rearrange("(r j) t -> r j t", j=JW)

            # ---------------- init ----------------
            # DMA issue order matters: the sync ring serializes dma_start at
            # ~650ns each, so wave-0-critical transfers go first.
            with nc.named_scope("init"):
                nc.sync.dma_start(out=QN[0][0:32, :, 1:TP1], in_=zlw_v[0:32, :, :])
                nc.sync.dma_start(out=QBX[0][0:32, 1:TP1], in_=qb0_in[:])
                nc.sync.dma_start(out=EB[0][0:32, :], in_=st0_in[:, 0:1])
                nc.sync.dma_start(out=DCOL[:], in_=dcol_in[:])
                # gpsimd: small wave-0-critical memsets first; big QN[0] zeros
                # chunked (long gpsimd ops stall concurrent DVE ops)
                nc.gpsimd.memset(B0[1][:], 0.0)
                nc.gpsimd.memset(B0[1][:, TSEG:TP1], 1.0)
                nc.gpsimd.memset(Z33[:], 0.0)
                nc.gpsimd.memset(Scol[:], 0.0)
                nc.gpsimd.memset(rcol[:], 1.0)
                # rows 0:32 of QBX[0] come from the host DMA issued above —
                # zero only the inactive rows (program order would otherwise
                # let this memset clobber the DMA'd data)
                nc.gpsimd.memset(QBX[0][32:64, :], 0.0)
                nc.gpsimd.memset(QBX[0][64:128, :], 0.0)
                for w in range(NCH):
                    nc.gpsimd.memset(QBX[w][:, 0:1], 1.0)
                nc.gpsimd.memset(CURL[1][:, :, TP1:TP2], 0.0)
                nc.gpsimd.memset(CURB[1][:, :, TSEG:TP1], 0.0)
                # wave-0 u_0 rows 32:128 are never written (no stt at j=0):
                # zero them once so stale NaN bit patterns can't leak in
                nc.gpsimd.memset(UT[32:64, 0, 1:TP1], 0.0)
                nc.gpsimd.memset(UT[64:128, 0, 1:TP1], 0.0)
                nc.gpsimd.memset(QN[0][:, :, 0:1], 1.0)
                for a, b in ((32, 64), (64, 128)):
                    for jj in range(0, JW, 8):
                        nc.gpsimd.memset(QN[0][a:b, jj:jj + 8, :], 0.0)
                # remaining host seg-0 stats + non-critical loads
                for q in range(1, NCH):
                    rows = slice(32 * q, 32 * q + 32)
                    nc.sync.dma_start(out=QBX[q][rows, 1:TP1], in_=qb0_in[:])
                    nc.sync.dma_start(out=EB[q][rows, :], in_=st0_in[:, 0:1])
                nc.sync.dma_start(out=SHI[:], in_=shiftm[:, 0:128])
                nc.sync.dma_start(out=SHFF[:], in_=shiftm[:, 128:256])
                nc.sync.dma_start(out=ZBK[:], in_=zbk[:])
                nc.sync.dma_start(out=LL[:, 0:1], in_=st0_in[:, 1:2])
                nc.sync.dma_start(out=LL[:, NSEG:NSEG + 1], in_=st0_in[:, 2:3])
                nc.vector.tensor_copy(out=SHF[:], in_=SHFF[:])
                make_identity(nc, IDENT[:])

            def late_memsets(w):
                # zero tiles first touched at wave w+1+, off the startup path.
                # Only rows never written by emit/stats need zeroing; chunks
                # stay small so concurrent DVE ops don't stall on the port.
                if w == 0:
                    for jj in range(0, JW, 8):
                        nc.gpsimd.memset(QN[1][64:128, jj:jj + 8, :], 0.0)
                    nc.gpsimd.memset(QN[1][:, :, 0:1], 1.0)
                    # inactive rows only (other rows carry host/stats data)
                    nc.gpsimd.memset(QBX[1][64:128, :], 0.0)
                    nc.gpsimd.memset(QBX[1][64:128, 0:1], 1.0)
                    nc.gpsimd.memset(QBX[2][96:128, :], 0.0)
                    nc.gpsimd.memset(QBX[2][96:128, 0:1], 1.0)
                    nc.gpsimd.memset(QBX[4][0:32, :], 0.0)
                    nc.gpsimd.memset(QBX[4][:, 0:1], 1.0)
                elif w == 1:
                    for jj in range(0, JW, 8):
                        nc.gpsimd.memset(QN[2][96:128, jj:jj + 8, :], 0.0)
                    nc.gpsimd.memset(QN[2][:, :, 0:1], 1.0)
                    nc.gpsimd.memset(QBX[5][0:64, :], 0.0)
                    nc.gpsimd.memset(QBX[5][:, 0:1], 1.0)
                    nc.gpsimd.memset(QBX[6][0:96, :], 0.0)
                    nc.gpsimd.memset(QBX[6][:, 0:1], 1.0)

            # ---------------- per-segment stats ----------------
            ydN_w = ydN.rearrange("(b k t) c -> k t b c", b=BLOC, k=NSEG)

            def stats(k):
                with nc.named_scope(f"stats{k}"):
                    pbt = workp.tile([BLOC, TSEG], f32, name="pbt", tag="pbt")
                    gk = workp.tile([BLOC, TSEG], f32, name="gk", tag="gk")
                    scr = workp.tile([BLOC, TSEG], f32, name="scr", tag="scr")
                    scr2 = workp.tile([BLOC, TSEG], f32, name="scr2", tag="scr2")
                    lgc = workp.tile([BLOC, 1], f32, name="lgc", tag="lgc")
                    qbkh = workp.tile([BLOC, TSEG], bf16, name="qbkh", tag="qbkh")
                    denp = psum.tile([BLOC, TSEG], f32, name="denp", tag="denp")
                    nc.scalar.activation(out=pbt[:], in_=ZBK[:, k * TSEG:(k + 1) * TSEG],
                                         func=Act.Exp)
                    for b0 in range(0, BLOC, 4):
                        zt = loadp.tile([TSEG, 4, C], f32, name="zt", tag="zt")
                        pt = loadp.tile([TSEG, 4, C], f32, name="pt", tag="pt")
                        nc.sync.dma_start(out=zt[:], in_=ydN_w[k, :, b0:b0 + 4, :])
                        for bi in range(4):
                            b = b0 + bi
                            nc.scalar.activation(out=pt[:, bi, :], in_=zt[:, bi, :],
                                                 func=Act.Exp,
                                                 accum_out=SDEN[:, 32 * k + b:32 * k + b + 1])
                    nc.tensor.transpose(out=denp[:], in_=SDEN[:, 32 * k:32 * k + 32],
                                        identity=IDENT[:])
                    # g = den/C + pb ; ln g (accum -> LOGG). Normalize with the
                    # per-(example, segment) CONSTANT c = exp(-LOGG/TSEG)
                    # instead of per-t 1/g: c^TSEG = exp(-LOGG) so the total
                    # path normalization (and the host formula) is unchanged,
                    # and the emit multiply becomes the ACT exp's bias.
                    nc.vector.scalar_tensor_tensor(
                        out=gk[:], in0=denp[:], scalar=1.0 / C, in1=pbt[:],
                        op0=Alu.mult, op1=Alu.add)
                    nc.scalar.activation(out=scr[:], in_=gk[:], func=Act.Ln,
                                         accum_out=LL[:, k:k + 1])
                    nc.vector.tensor_scalar(out=lgc[:], in0=LL[:, k:k + 1],
                                            scalar1=-1.0 / TSEG, scalar2=0.0,
                                            op0=Alu.mult, op1=Alu.add)
                    nc.scalar.activation(out=scr2[:], in_=denp[:], func=Act.Ln,
                                         accum_out=LL[:, NSEG + k:NSEG + k + 1])
                    # qb' = pb * c = exp(zbk + ln c)
                    nc.scalar.activation(out=qbkh[:],
                                         in_=ZBK[:, k * TSEG:(k + 1) * TSEG],
                                         func=Act.Exp, bias=lgc[:, 0:1])
                    for q in range(NCH):
                        w = q + k
                        if w >= NW:
                            continue
                        rows = slice(32 * q, 32 * q + 32)
                        nc.sync.dma_start(out=QBX[w][rows, 1:TP1], in_=qbkh[:])
                        nc.sync.dma_start(out=EB[w][rows, :], in_=lgc[:])

            # ---------------- per-wave emit ----------------
            zlw_base = [0] * NW
            acc = 0
            for w in range(NW):
                zlw_base[w] = acc
                qlo, qhi, p0, p1 = _wave_ranges(w)
                acc += (qhi - qlo + 1) * 32 * JW
            zlw_v = zlw.rearrange("(r j) t -> r j t", j=JW)

            def emit_dma(w):
                P = w % 3
                qlo, qhi, p0, p1 = _wave_ranges(w)
                with nc.named_scope(f"emitd{w}"):
                    for a, b in qsplit(p0, p1):
                        r0 = zlw_base[w] // JW + (a - p0)
                        nc.sync.dma_start(
                            out=QN[P][a:b, :, 1:TP1],
                            in_=zlw_v[r0:r0 + (b - a), :, :],
                        )

            def emit_compute(w):
                # qn' = p * c = exp(zlw + ln c): the normalizer rides the ACT
                # bias, so no separate multiply op (gpsimd tensor ops stall
                # concurrent DVE ops on the shared port — keep gpsimd idle)
                P = w % 3
                qlo, qhi, p0, p1 = _wave_ranges(w)
                with nc.named_scope(f"emitc{w}"):
                    step = 8 if w == 0 else 16
                    for a, b in qsplit(p0, p1):
                        for jj in range(0, JW, step):
                            nc.scalar.activation(
                                out=QN[P][a:b, jj:jj + step, 1:TP1],
                                in_=QN[P][a:b, jj:jj + step, 1:TP1], func=Act.Exp,
                                bias=EB[w][a:b, 0:1])

            def emit(w):
                emit_dma(w)
                emit_compute(w)

            def b0_pre(w):
                # unrescaled b0 decay for wave w (issued under jloop(w-1)):
                # B0u[t] = B0o[TSEG] * prod qb; scaled by rcol at wstart(w).
                B0o = B0[1 - (w % 2)]
                nc.vector.tensor_tensor_scan(
                    out=B0u[:, 0:TP1], data0=Z33[:, 0:TP1],
                    data1=QBX[w][0:32, 0:TP1], initial=B0o[:, TSEG:TP1],
                    op0=Alu.add, op1=Alu.mult)

            # ---------------- waves ----------------
            # (wave-0 zlw DMA already issued at the top of init)
            emit_compute(0)
            b0_pre(0)

            for w in range(NW):
                P2 = w % 2
                P3 = w % 3
                qlo, qhi, p0, p1 = _wave_ranges(w)
                CL, CB = CURL[P2], CURB[P2]
                OL, OB = CURL[1 - P2], CURB[1 - P2]
                PB = PREVB[P2]
                B0c, B0o = B0[P2], B0[1 - P2]
                r1 = 32 * (min(NCH - 1, w - 1) + 1)

                with nc.named_scope(f"wstart{w}"):
                    if w >= 1:
                        # PE: shifted row-31 tiles + shifted Scol (old Scol)
                        psh = psum.tile([128, 1], f32, name="psh", tag="psh")
                        pshm = psum.tile([128, 1], f32, name="pshm", tag="pshm")
                        ppl = psum.tile([128, TSEG], f32, name="ppl", tag="ppl")
                        ppb = psum.tile([128, TSEG], f32, name="ppb", tag="ppb")
                        nc.tensor.matmul(out=psh[:], lhsT=SHI[:], rhs=Scol[:],
                                         start=True, stop=True)
                        nc.tensor.matmul(out=ppl[:], lhsT=SHF[:],
                                         rhs=OL[:, JW - 1, 1:TP1], start=True,
                                         stop=True)
                        nc.tensor.matmul(out=ppb[:], lhsT=SHF[:],
                                         rhs=OB[:, JW - 1, 0:TSEG], start=True,
                                         stop=True)
                        m31 = colp.tile([128, 1], f32, name="m31", tag="m31")
                        mt = colp.tile([128, 1], f32, name="mt", tag="mt")
                        m_own = colp.tile([128, 1], f32, name="m_own", tag="m_own")
                        m_in = colp.tile([128, 1], f32, name="m_in", tag="m_in")
                        lg1 = colp.tile([128, 1], f32, name="lg1", tag="lg1")
                        lg2 = colp.tile([128, 1], f32, name="lg2", tag="lg2")
                        peak = colp.tile([128, 1], f32, name="peak", tag="peak")
                        snew = colp.tile([128, 1], f32, name="snew", tag="snew")
                        dc = colp.tile([128, 1], f32, name="dc", tag="dc")
                        # incoming peak: max over row 31 of prev wave, shifted
                        nc.vector.tensor_reduce(out=m31[:], in_=OL[:, JW - 1, 1:TP2],
                                                axis=Ax.XY, op=Alu.max)
                        nc.vector.tensor_reduce(out=mt[:], in_=OB[:, JW - 1, 0:TP1],
                                                axis=Ax.XY, op=Alu.max)
                        nc.vector.tensor_tensor(out=m31[:], in0=m31[:], in1=mt[:],
                                                op=Alu.max)
                        nc.tensor.matmul(out=pshm[:], lhsT=SHFF[:], rhs=m31[:],
                                         start=True, stop=True)
                        # own peak: final column over all j
                        nc.vector.tensor_reduce(out=m_own[:],
                                                in_=OL[:, :, TP1:TP2],
                                                axis=Ax.XY, op=Alu.max)
                        nc.vector.tensor_reduce(out=lg1[:],
                                                in_=OB[:, :, TSEG:TP1],
                                                axis=Ax.XY, op=Alu.max)
                        nc.vector.tensor_tensor(out=m_own[:], in0=m_own[:],
                                                in1=lg1[:], op=Alu.max)
                        # the TINY floors are load-bearing: m=0 rows (chunks
                        # not yet active) would give rcol/corr = exp(125) =
                        # inf, and 0*inf = NaN poisons later chunks via the
                        # partition shift
                        nc.vector.tensor_scalar(out=m_own[:], in0=m_own[:],
                                                scalar1=TINY, scalar2=1e38,
                                                op0=Alu.max, op1=Alu.min)
                        # clamp fused into the PSUM read; the B0 override is
                        # clamped separately so every row stays >= TINY
                        nc.vector.tensor_scalar(out=m_in[:], in0=pshm[:],
                                                scalar1=TINY, scalar2=1e38,
                                                op0=Alu.max, op1=Alu.min)
                        if qlo == 0:
                            nc.vector.tensor_scalar(out=m_in[0:32, :],
                                                    in0=B0o[:, TSEG:TP1],
                                                    scalar1=TINY, scalar2=1e38,
                                                    op0=Alu.max, op1=Alu.min)
                        # lg = ln(m) via exponent-bits approximation: a single
                        # tensor_copy from the int32 view VALUE-converts to
                        # f32 (ALU ops would treat the bits raw, copies don't)
                        nc.vector.tensor_copy(out=lg1[:],
                                              in_=m_own[:].bitcast(i32))
                        nc.vector.tensor_scalar(out=lg1[:], in0=lg1[:],
                                                scalar1=KLN, scalar2=-FB * KLN,
                                                op0=Alu.mult, op1=Alu.add)
                        nc.vector.tensor_copy(out=lg2[:],
                                              in_=m_in[:].bitcast(i32))
                        nc.vector.tensor_scalar(out=lg2[:], in0=lg2[:],
                                                scalar1=KLN, scalar2=-FB * KLN,
                                                op0=Alu.mult, op1=Alu.add)
                        nc.vector.tensor_tensor(out=lg1[:], in0=lg1[:],
                                                in1=Scol[:], op=Alu.subtract)
                        nc.vector.tensor_tensor(out=lg2[:], in0=lg2[:],
                                                in1=psh[:], op=Alu.subtract)
                        nc.vector.tensor_tensor(out=peak[:], in0=lg1[:],
                                                in1=lg2[:], op=Alu.max)
                        nc.vector.tensor_scalar(out=snew[:], in0=peak[:],
                                                scalar1=-1.0, scalar2=LT,
                                                op0=Alu.mult, op1=Alu.add)
                        # rcol = exp(snew - Scol_old); corr = exp(Scol_new - Sshift)
                        nc.vector.tensor_tensor(out=ARGS[:, 0:1], in0=snew[:],
                                                in1=Scol[:], op=Alu.subtract)
                        for a, b in qsplit(p0, r1):
                            nc.vector.tensor_copy(out=Scol[a:b, :], in_=snew[a:b, :])
                        nc.vector.tensor_tensor(out=ARGS[:, 1:2], in0=Scol[:],
                                                in1=psh[:], op=Alu.subtract)
                        nc.scalar.activation(out=RCE[:], in_=ARGS[:], func=Act.Exp)
                        # prev-row handoff at incoming scale correction; the
                        # j=0 stt is fused here: U0 = (d*corr)*ppl + ppb*corr
                        nc.vector.tensor_scalar_mul(PB[:], ppb[:], RCE[:, 1:2])
                        nc.vector.tensor_tensor(out=dc[:], in0=DCOL[:, 0:1],
                                                in1=RCE[:, 1:2], op=Alu.mult)
                        nc.vector.scalar_tensor_tensor(
                            out=UT[:, 0, 1:TP1], in0=ppl[:], scalar=dc[:, 0:1],
                            in1=PB[:], op0=Alu.mult, op1=Alu.add)
                    rc = RCE[:, 0:1] if w >= 1 else rcol[:, 0:1]
                    if w <= NSEG - 1:
                        # chunk-0 row: b0 decay, rescaled (only the final
                        # column of B0c is ever read); u_0 rows 0:32 = b0 row
                        nc.vector.tensor_scalar_mul(B0c[:, TSEG:TP1],
                                                    B0u[:, TSEG:TP1],
                                                    rc[0:32, :])
                        nc.vector.tensor_scalar_mul(UT[0:32, 0, 1:TP1],
                                                    B0u[:, None, 0:TSEG],
                                                    rc[0:32, :])
                    # boundary handoff into this wave's tiles (DVE: cheap and
                    # keeps the jloop's first scans same-engine-ordered)
                    nc.vector.tensor_scalar_mul(UT[:, :, 0:1], OL[:, :, TP1:TP2],
                                                rc[:, :])
                    nc.vector.tensor_scalar_mul(CL[:, :, 0:1], OB[:, :, TSEG:TP1],
                                                rc[:, :])

                with nc.named_scope(f"jloop{w}"):
                    for j in range(JW):
                        if j > 0:  # j=0's stt is fused into wstart
                            nc.vector.scalar_tensor_tensor(
                                out=UT[:, j, 1:TP1], in0=CL[:, j - 1, 1:TP1],
                                scalar=DCOL[:, j:j + 1], in1=CB[:, j - 1, 0:TSEG],
                                op0=Alu.mult, op1=Alu.add)
                        nc.vector.tensor_tensor_scan(
                            out=CL[:, j, 1:TP2], data0=UT[:, j, 0:TP1],
                            data1=QN[P3][:, j, 0:TP1], initial=0.0,
                            op0=Alu.add, op1=Alu.mult)
                        nc.vector.tensor_tensor_scan(
                            out=CB[:, j, 0:TP1], data0=CL[:, j, 0:TP1],
                            data1=QBX[w][:, 0:TP1], initial=0.0,
                            op0=Alu.add, op1=Alu.mult)

                if w >= NSEG - 1:  # chunk q = w - (NSEG-1) finished segment 3
                    q = w - (NSEG - 1)
                    rows = slice(32 * q, 32 * q + 32)
                    nc.vector.tensor_copy(out=STOUT[rows, 0:JW],
                                          in_=CL[rows, :, TP1:TP2])
                    nc.vector.tensor_copy(out=STOUT[rows, JW:2 * JW],
                                          in_=CB[rows, :, TSEG:TP1])

                # next wave's stats/emit/b0 run under this wave's jloop
                late_memsets(w)
                if w + 1 < NW:
                    if w + 1 <= NSEG - 1:
                        stats(w + 1)
                    emit(w + 1)
                    if w + 1 <= NSEG - 1:
                        b0_pre(w + 1)

            nc.vector.tensor_copy(out=STOUT[:, 2 * JW:2 * JW + 1], in_=Scol[:])
            nc.sync.dma_start(out=out_st[:], in_=STOUT[:])
            nc.sync.dma_start(out=out_ll[:], in_=LL[:])
    if split_waits:
        _split_waits(nc)
    return nc


def host_prep(y_true, y_pred):
    """Build per-core input maps."""
    import ml_dtypes

    y_true = np.asarray(y_true).astype(np.int32)
    y_pred = np.asarray(y_pred).astype(np.float32)
    shiftm = np.zeros((128, 256), np.float32)
    for m in range(128):
        shiftm[m - 32 if m >= 32 else m, m] = 1.0          # SHI
        if m >= 32:
            shiftm[m - 32, 128 + m] = 1.0                  # SHF
    in_maps = []
    for core in range(NCORE):
        yt = y_true[core * BLOC:(core + 1) * BLOC]        # [32, 128]
        yp = y_pred[core * BLOC:(core + 1) * BLOC]        # [32, 512, 128]
        ydN = np.ascontiguousarray(yp).reshape(BLOC * NSEG * TSEG, C)
        zbk = np.ascontiguousarray(yp[:, :, BLANK])       # [32, 512]
        # gathered label logits, wave-block layout
        zg = np.take_along_axis(yp, yt.astype(np.int64)[:, None, :], axis=2)
        zgt = zg.transpose(0, 2, 1)                       # [32 b, 128 i, 512 t]
        zlw = np.zeros((512 * JW, TSEG), np.float32)
        base = 0
        for w in range(NW):
            qlo, qhi, p0, p1 = _wave_ranges(w)
            for q in range(qlo, qhi + 1):
                k = w - q
                blk = zgt[:, 32 * q:32 * q + JW, k * TSEG:(k + 1) * TSEG]
                n = BLOC * JW
                zlw[base:base + n] = blk.reshape(n, TSEG)
                base += n
        zlw = zlw.astype(ml_dtypes.bfloat16)
        dcol = np.zeros((128, JW), np.float32)
        for q in range(NCH):
            for j in range(JW):
                i = 32 * q + j + 1
                if i >= 2:
                    dcol[32 * q:32 * q + 32, j] = (
                        yt[:, i - 1] != yt[:, i - 2]).astype(np.float32)
        # segment-0 stats (device computes segments 1-3 under the DP waves)
        yp0 = yp[:, 0:TSEG, :].astype(np.float64)
        den0 = np.exp(yp0).sum(-1)                         # [32, 128]
        pb0 = np.exp(yp0[:, :, BLANK])
        g0 = pb0 + den0 / C
        logg0 = np.log(g0).sum(1)
        logden0 = np.log(den0).sum(1)
        lgc0 = -logg0 / TSEG
        qb0 = np.exp(yp0[:, :, BLANK] + lgc0[:, None]).astype(ml_dtypes.bfloat16)
        st0 = np.stack([lgc0, logg0, logden0], 1).astype(np.float32)
        in_maps.append({"zlw": zlw, "ydN": ydN, "zbk": zbk, "dcol": dcol,
                        "shiftm": shiftm, "qb0": qb0, "st0": st0})
    return in_maps


def host_finish(y_true, results):
    y_true = np.asarray(y_true)
    ll = (y_true != 0).sum(axis=1).astype(np.int64)        # [256]
    losses = np.zeros(B, np.float64)
    for core in range(NCORE):
        res = results[core]
        st = np.asarray(res["out_st"], dtype=np.float64)
        tl, tb, S = st[:, 0:JW], st[:, JW:2 * JW], st[:, 2 * JW:2 * JW + 1]
        ll2 = np.asarray(res["out_ll"])
        logg, logden = ll2[:, 0:NSEG], ll2[:, NSEG:2 * NSEG]
        for b in range(BLOC):
            gb = core * BLOC + b
            l = int(ll[gb])
            q, j = (l - 1) // 32, (l - 1) % 32
            p = 32 * q + b
            logP = np.log(tl[p, j] + tb[p, j])
            losses[gb] = -(logP + logg[b].sum() - logden[b].sum() - S[p, 0])
    return np.float32(losses.mean())


def _get_program():
    if "nc" not in _PROG:
        _PROG["nc"] = build_program()
    return _PROG["nc"]


def kernel(y_true: np.ndarray, y_pred: np.ndarray) -> np.ndarray:
    from concourse.bass_utils import run_bass_kernel_spmd

    nc = _get_program()
    in_maps = host_prep(y_true, y_pred)
    res = run_bass_kernel_spmd(nc, in_maps, core_ids=list(range(NCORE)))
    return host_finish(y_true, res.results)


# revision 13
# speedup vs baseline: 1.0013x; 1.0013x over previous
"""CTC mean loss on 8 trn2 NeuronCores (Bass/Tile) — restructured.

Per example: linear-domain CTC forward DP with per-t normalizer
g = pb + den/C. Label-position recurrences run as tensor_tensor_scan
over time; 4 label chunks x 4 time segments wavefront over 128
partitions, with per-wave boundary rescaling (log-scale tracked in Scol).

Layout (vs the previous version): CURL[., j, 0] holds the b-boundary so
the CB scan runs with an immediate 0.0 initial (AP initials cost ~86ns
extra per scan); CURL[., j, 1+t] = l_j[t].  Stats for segment k and the
emit (gathered-logit exp) for wave w+1 are issued behind jloop(w) and
run on ACT/DMA/gpsimd under the DVE wave grind.  The rescale chain uses
ACT exp instead of exact exp2 bitcast round-trips (S absorbs the tiny
log error), and outputs are staged into contiguous tiles and shipped
once at the end.
"""
import numpy as np

# problem constants (fixed by the spec)
B, T, C, L = 256, 512, 128, 128
NCORE = 8
BLOC = B // NCORE          # 32 examples per core
NCH, JW = 4, 32            # label chunks x positions per chunk (i = 32q+j+1)
NSEG, TSEG = 4, 128        # time segments
NW = NSEG + NCH - 1        # 7 waves
BLANK = C - 1
LT = 38.0                  # log rescale target
TINY = 4e-18
KLN = float(np.log(2.0) / (1 << 23))
FB = float(127 << 23)

_PROG = {}


def _wave_ranges(w):
    qlo, qhi = max(0, w - (NSEG - 1)), min(NCH - 1, w)
    return qlo, qhi, 32 * qlo, 32 * (qhi + 1)


MAXEND = {0: 128, 32: 64, 64: 128, 96: 128}


def qsplit(a, b):
    """Decompose [a, b) into SBUF-legal partition ranges (quadrant rules)."""
    out = []
    while a < b:
        e = min(b, MAXEND[a])
        if e <= a:
            e = min(c for c in (32, 64, 96, 128) if c > a)
        out.append((a, e))
        a = e
    return out


def _patch_drain():
    """This container's walrus rejects TPB_CTRL drains with >1 sem wait;
    split the TileContext exit drain into one-wait-per-drain instructions."""
    import bass_rust
    import concourse.tile as tile_mod
    from concourse.tile import ScopedClock

    if getattr(tile_mod.TileContext, "_drain_split_patch", False):
        return

    def patched(self, tick_clock, wait_clock):
        drain_inst = self.nc.sync.drain()
        wait_clock.add_sem_waits(
            drain_inst.ins, ScopedClock({None: tick_clock.global_clock})
        )
        si = drain_inst.ins.sync_info
        waits = list(si.on_wait) if si is not None else []
        if len(waits) > 1:
            drain_inst.ins.sync_info = bass_rust.SyncInfo(
                on_wait=waits[:1], on_update=list(si.on_update)
            )
            for i in range(1, len(waits)):
                extra = self.nc.sync.drain()
                extra.ins.sync_info = bass_rust.SyncInfo(
                    on_wait=[waits[i]], on_update=[]
                )
        self.nc.all_engine_barrier()
        popped = self.nc._tile_sem_poison_stack.pop()
        assert popped is self._sem_poison
        self.nc.clear_and_free_semaphores(list(self.sems.allocated().values()))
        self.nc.all_engine_barrier()

    tile_mod.TileContext._drain_and_barrier = patched
    tile_mod.TileContext._drain_split_patch = True


def _split_waits(nc):
    """This container's walrus accepts at most ONE sem wait per instruction;
    hoist extra waits onto same-engine NoOps inserted just before."""
    import bass_rust

    cnt = 0
    for f in nc.m.functions:
        for bb in f.blocks:
            new = []
            changed = False
            for inst in bb.instructions:
                si = inst.sync_info
                waits = list(si.on_wait) if si is not None else []
                if len(waits) > 1:
                    changed = True
                    for wt in waits[:-1]:
                        cnt += 1
                        nop = bass_rust.InstNoOp(
                            name=f"I-wsplit-{cnt}", engine=inst.engine
                        )
                        nop.sync_info = bass_rust.SyncInfo(on_wait=[wt], on_update=[])
                        new.append(nop)
                    inst.sync_info = bass_rust.SyncInfo(
                        on_wait=[waits[-1]], on_update=list(si.on_update)
                    )
                new.append(inst)
            if changed:
                bb.instructions = new
    return cnt


def build_program(split_waits=True):
    import concourse.bass as bass
    import concourse.mybir as mybir
    from concourse.tile import TileContext
    from concourse.masks import make_identity

    _patch_drain()
    f32 = mybir.dt.float32
    bf16 = mybir.dt.bfloat16
    i32 = mybir.dt.int32
    Alu = mybir.AluOpType
    Act = mybir.ActivationFunctionType
    Ax = mybir.AxisListType

    nc = bass.Bass()
    shiftm = nc.declare_dram_parameter("shiftm", [128, 256], f32, isOutput=False)
    zlw = nc.declare_dram_parameter("zlw", [512 * JW, TSEG], bf16, isOutput=False)
    ydN = nc.declare_dram_parameter("ydN", [BLOC * NSEG * TSEG, C], f32, isOutput=False)
    zbk = nc.declare_dram_parameter("zbk", [BLOC, NSEG * TSEG], f32, isOutput=False)
    dcol_in = nc.declare_dram_parameter("dcol", [128, JW], f32, isOutput=False)
    # segment-0 stats precomputed host-side (waves 0..3 need them at start;
    # segments 1-3 are computed on-device, overlapped under the DP waves)
    qb0_in = nc.declare_dram_parameter("qb0", [BLOC, TSEG], bf16, isOutput=False)
    st0_in = nc.declare_dram_parameter("st0", [BLOC, 3], f32, isOutput=False)
    out_st = nc.declare_dram_parameter("out_st", [128, 2 * JW + 1], f32,
                                       isOutput=True)
    out_ll = nc.declare_dram_parameter("out_ll", [BLOC, 2 * NSEG], f32,
                                       isOutput=True)

    TP2 = TSEG + 2   # CURL free width per j: [bbnd, l_0 .. l_TSEG]
    TP1 = TSEG + 1

    with TileContext(nc) as tc:
        with (
            tc.tile_pool(name="pers", bufs=1) as pers,
            tc.tile_pool(name="psum", bufs=1, space="PSUM") as psum,
            tc.tile_pool(name="load", bufs=6) as loadp,
            tc.tile_pool(name="work", bufs=6) as workp,
            tc.tile_pool(name="cols", bufs=2) as colp,
        ):
            # ---------------- persistent state ----------------
            CURL = [pers.tile([128, JW, TP2], bf16, name=f"curl{p}", tag=f"curl{p}")
                    for p in range(2)]
            CURB = [pers.tile([128, JW, TP1], bf16, name=f"curb{p}", tag=f"curb{p}")
                    for p in range(2)]
            QN = [pers.tile([128, JW, TP1], bf16, name=f"qn{p}", tag=f"qn{p}")
                  for p in range(3)]
            UT = pers.tile([128, JW, TP1], bf16, name="ut", tag="ut")
            QBX = [pers.tile([128, TP1], bf16, name=f"qbx{w}", tag=f"qbx{w}")
                   for w in range(NW)]
            EB = [pers.tile([128, 1], f32, name=f"eb{w}", tag=f"eb{w}")
                  for w in range(NW)]
            PREVB = [pers.tile([128, TSEG], bf16, name=f"prevb{p}", tag=f"prevb{p}")
                     for p in range(2)]
            B0 = [pers.tile([32, TP1], f32, name=f"b0{p}", tag=f"b0{p}")
                  for p in range(2)]
            B0u = pers.tile([32, TP1], f32, name="b0u", tag="b0u")
            Z33 = pers.tile([32, TP1], f32, name="z33", tag="z33")
            DCOL = pers.tile([128, JW], f32, name="dcol", tag="dcol")
            IDENT = pers.tile([128, 128], f32, name="ident", tag="ident")
            SHI = pers.tile([128, 128], f32, name="shi", tag="shi")
            SHFF = pers.tile([128, 128], f32, name="shff", tag="shff")
            SHF = pers.tile([128, 128], bf16, name="shf", tag="shf")
            SDEN = pers.tile([128, NSEG * BLOC], f32, name="sden", tag="sden")
            ZBK = pers.tile([BLOC, NSEG * TSEG], f32, name="zbkt", tag="zbkt")
            LL = pers.tile([BLOC, 2 * NSEG], f32, name="ll", tag="ll")
            STOUT = pers.tile([128, 2 * JW + 1], f32, name="stout", tag="stout")
            Scol = pers.tile([128, 1], f32, name="scol", tag="scol")
            rcol = pers.tile([128, 1], f32, name="rcol", tag="rcol")
            ARGS = pers.tile([128, 2], f32, name="args", tag="args")
            RCE = pers.tile([128, 2], f32, name="rce", tag="rce")

            zlw_v = zlw.rearrange("(r j) t -> r j t", j=JW)

            # ---------------- init ----------------
            # DMA issue order matters: the sync ring serializes dma_start at
            # ~650ns each, so wave-0-critical transfers go first.
            with nc.named_scope("init"):
                nc.sync.dma_start(out=QN[0][0:32, :, 1:TP1], in_=zlw_v[0:32, :, :])
                nc.sync.dma_start(out=QBX[0][0:32, 1:TP1], in_=qb0_in[:])
                nc.sync.dma_start(out=EB[0][0:32, :], in_=st0_in[:, 0:1])
                nc.sync.dma_start(out=DCOL[:], in_=dcol_in[:])
                # gpsimd: small wave-0-critical memsets first; big QN[0] zeros
                # chunked (long gpsimd ops stall concurrent DVE ops)
                nc.gpsimd.memset(B0[1][:], 0.0)
                nc.gpsimd.memset(B0[1][:, TSEG:TP1], 1.0)
                nc.gpsimd.memset(Z33[:], 0.0)
                nc.gpsimd.memset(Scol[:], 0.0)
                nc.gpsimd.memset(rcol[:], 1.0)
                # rows 0:32 of QBX[0] come from the host DMA issued above —
                # zero only the inactive rows (program order would otherwise
                # let this memset clobber the DMA'd data)
                nc.gpsimd.memset(QBX[0][32:64, :], 0.0)
                nc.gpsimd.memset(QBX[0][64:128, :], 0.0)
                for w in range(NCH):
                    nc.gpsimd.memset(QBX[w][:, 0:1], 1.0)
                nc.gpsimd.memset(CURL[1][:, :, TP1:TP2], 0.0)
                nc.gpsimd.memset(CURB[1][:, :, TSEG:TP1], 0.0)
                # wave-0 u_0 rows 32:128 are never written (no stt at j=0):
                # zero them once so stale NaN bit patterns can't leak in
                nc.gpsimd.memset(UT[32:64, 0, 1:TP1], 0.0)
                nc.gpsimd.memset(UT[64:128, 0, 1:TP1], 0.0)
                nc.gpsimd.memset(QN[0][:, :, 0:1], 1.0)
                for a, b in ((32, 64), (64, 128)):
                    for jj in range(0, JW, 8):
                        nc.gpsimd.memset(QN[0][a:b, jj:jj + 8, :], 0.0)
                # remaining host seg-0 stats + non-critical loads
                for q in range(1, NCH):
                    rows = slice(32 * q, 32 * q + 32)
                    nc.sync.dma_start(out=QBX[q][rows, 1:TP1], in_=qb0_in[:])
                    nc.sync.dma_start(out=EB[q][rows, :], in_=st0_in[:, 0:1])
                nc.sync.dma_start(out=SHI[:], in_=shiftm[:, 0:128])
                nc.sync.dma_start(out=SHFF[:], in_=shiftm[:, 128:256])
                nc.sync.dma_start(out=ZBK[:], in_=zbk[:])
                nc.sync.dma_start(out=LL[:, 0:1], in_=st0_in[:, 1:2])
                nc.sync.dma_start(out=LL[:, NSEG:NSEG + 1], in_=st0_in[:, 2:3])
                nc.vector.tensor_copy(out=SHF[:], in_=SHFF[:])
                make_identity(nc, IDENT[:])

            def late_memsets(w):
                # zero tiles first touched at wave w+1+, off the startup path.
                # Only rows never written by emit/stats need zeroing; chunks
                # stay small so concurrent DVE ops don't stall on the port.
                if w == 0:
                    for jj in range(0, JW, 8):
                        nc.gpsimd.memset(QN[1][64:128, jj:jj + 8, :], 0.0)
                    nc.gpsimd.memset(QN[1][:, :, 0:1], 1.0)
                    # inactive rows only (other rows carry host/stats data)
                    nc.gpsimd.memset(QBX[1][64:128, :], 0.0)
                    nc.gpsimd.memset(QBX[1][64:128, 0:1], 1.0)
                    nc.gpsimd.memset(QBX[2][96:128, :], 0.0)
                    nc.gpsimd.memset(QBX[2][96:128, 0:1], 1.0)
                    nc.gpsimd.memset(QBX[4][0:32, :], 0.0)
                    nc.gpsimd.memset(QBX[4][:, 0:1], 1.0)
                elif w == 1:
                    for jj in range(0, JW, 8):
                        nc.gpsimd.memset(QN[2][96:128, jj:jj + 8, :], 0.0)
                    nc.gpsimd.memset(QN[2][:, :, 0:1], 1.0)
                    nc.gpsimd.memset(QBX[5][0:64, :], 0.0)
                    nc.gpsimd.memset(QBX[5][:, 0:1], 1.0)
                    nc.gpsimd.memset(QBX[6][0:96, :], 0.0)
                    nc.gpsimd.memset(QBX[6][:, 0:1], 1.0)

            # ---------------- per-segment stats ----------------
            ydN_w = ydN.rearrange("(b k t) c -> k t b c", b=BLOC, k=NSEG)

            def stats(k):
                with nc.named_scope(f"stats{k}"):
                    pbt = workp.tile([BLOC, TSEG], f32, name="pbt", tag="pbt")
                    gk = workp.tile([BLOC, TSEG], f32, name="gk", tag="gk")
                    scr = workp.tile([BLOC, TSEG], f32, name="scr", tag="scr")
                    scr2 = workp.tile([BLOC, TSEG], f32, name="scr2", tag="scr2")
                    lgc = workp.tile([BLOC, 1], f32, name="lgc", tag="lgc")
                    qbkh = workp.tile([BLOC, TSEG], bf16, name="qbkh", tag="qbkh")
                    denp = psum.tile([BLOC, TSEG], f32, name="denp", tag="denp")
                    nc.scalar.activation(out=pbt[:], in_=ZBK[:, k * TSEG:(k + 1) * TSEG],
                                         func=Act.Exp)
                    for b0 in range(0, BLOC, 4):
                        zt = loadp.tile([TSEG, 4, C], f32, name="zt", tag="zt")
                        pt = loadp.tile([TSEG, 4, C], f32, name="pt", tag="pt")
                        nc.sync.dma_start(out=zt[:], in_=ydN_w[k, :, b0:b0 + 4, :])
                        for bi in range(4):
                            b = b0 + bi
                            nc.scalar.activation(out=pt[:, bi, :], in_=zt[:, bi, :],
                                                 func=Act.Exp,
                                                 accum_out=SDEN[:, 32 * k + b:32 * k + b + 1])
                    nc.tensor.transpose(out=denp[:], in_=SDEN[:, 32 * k:32 * k + 32],
                                        identity=IDENT[:])
                    # g = den/C + pb ; ln g (accum -> LOGG). Normalize with the
                    # per-(example, segment) CONSTANT c = exp(-LOGG/TSEG)
                    # instead of per-t 1/g: c^TSEG = exp(-LOGG) so the total
                    # path normalization (and the host formula) is unchanged,
                    # and the emit multiply becomes the ACT exp's bias.
                    nc.vector.scalar_tensor_tensor(
                        out=gk[:], in0=denp[:], scalar=1.0 / C, in1=pbt[:],
                        op0=Alu.mult, op1=Alu.add)
                    nc.scalar.activation(out=scr[:], in_=gk[:], func=Act.Ln,
                                         accum_out=LL[:, k:k + 1])
                    nc.vector.tensor_scalar(out=lgc[:], in0=LL[:, k:k + 1],
                                            scalar1=-1.0 / TSEG, scalar2=0.0,
                                            op0=Alu.mult, op1=Alu.add)
                    nc.scalar.activation(out=scr2[:], in_=denp[:], func=Act.Ln,
                                         accum_out=LL[:, NSEG + k:NSEG + k + 1])
                    # qb' = pb * c = exp(zbk + ln c)
                    nc.scalar.activation(out=qbkh[:],
                                         in_=ZBK[:, k * TSEG:(k + 1) * TSEG],
                                         func=Act.Exp, bias=lgc[:, 0:1])
                    for q in range(NCH):
                        w = q + k
                        if w >= NW:
                            continue
                        rows = slice(32 * q, 32 * q + 32)
                        nc.sync.dma_start(out=QBX[w][rows, 1:TP1], in_=qbkh[:])
                        nc.sync.dma_start(out=EB[w][rows, :], in_=lgc[:])

            # ---------------- per-wave emit ----------------
            zlw_base = [0] * NW
            acc = 0
            for w in range(NW):
                zlw_base[w] = acc
                qlo, qhi, p0, p1 = _wave_ranges(w)
                acc += (qhi - qlo + 1) * 32 * JW
            zlw_v = zlw.rearrange("(r j) t -> r j t", j=JW)

            def emit_dma(w):
                P = w % 3
                qlo, qhi, p0, p1 = _wave_ranges(w)
                with nc.named_scope(f"emitd{w}"):
                    for a, b in qsplit(p0, p1):
                        r0 = zlw_base[w] // JW + (a - p0)
                        nc.sync.dma_start(
                            out=QN[P][a:b, :, 1:TP1],
                            in_=zlw_v[r0:r0 + (b - a), :, :],
                        )

            def emit_compute(w):
                # qn' = p * c = exp(zlw + ln c): the normalizer rides the ACT
                # bias, so no separate multiply op (gpsimd tensor ops stall
                # concurrent DVE ops on the shared port — keep gpsimd idle)
                P = w % 3
                qlo, qhi, p0, p1 = _wave_ranges(w)
                with nc.named_scope(f"emitc{w}"):
                    for a, b in qsplit(p0, p1):
                        for jj in range(0, JW, 16):
                            nc.scalar.activation(
                                out=QN[P][a:b, jj:jj + 16, 1:TP1],
                                in_=QN[P][a:b, jj:jj + 16, 1:TP1], func=Act.Exp,
                                bias=EB[w][a:b, 0:1])

            def emit(w):
                emit_dma(w)
                emit_compute(w)

            def b0_pre(w):
                # unrescaled b0 decay for wave w (issued under jloop(w-1)):
                # B0u[t] = B0o[TSEG] * prod qb; scaled by rcol at wstart(w).
                B0o = B0[1 - (w % 2)]
                nc.vector.tensor_tensor_scan(
                    out=B0u[:, 0:TP1], data0=Z33[:, 0:TP1],
                    data1=QBX[w][0:32, 0:TP1], initial=B0o[:, TSEG:TP1],
                    op0=Alu.add, op1=Alu.mult)

            # ---------------- waves ----------------
            # (wave-0 zlw DMA already issued at the top of init)
            emit_compute(0)
            b0_pre(0)

            for w in range(NW):
                P2 = w % 2
                P3 = w % 3
                qlo, qhi, p0, p1 = _wave_ranges(w)
                CL, CB = CURL[P2], CURB[P2]
                OL, OB = CURL[1 - P2], CURB[1 - P2]
                PB = PREVB[P2]
                B0c, B0o = B0[P2], B0[1 - P2]
                r1 = 32 * (min(NCH - 1, w - 1) + 1)

                with nc.named_scope(f"wstart{w}"):
                    if w >= 1:
                        # PE: shifted row-31 tiles + shifted Scol (old Scol)
                        psh = psum.tile([128, 1], f32, name="psh", tag="psh")
                        pshm = psum.tile([128, 1], f32, name="pshm", tag="pshm")
                        ppl = psum.tile([128, TSEG], f32, name="ppl", tag="ppl")
                        ppb = psum.tile([128, TSEG], f32, name="ppb", tag="ppb")
                        nc.tensor.matmul(out=psh[:], lhsT=SHI[:], rhs=Scol[:],
                                         start=True, stop=True)
                        nc.tensor.matmul(out=ppl[:], lhsT=SHF[:],
                                         rhs=OL[:, JW - 1, 1:TP1], start=True,
                                         stop=True)
                        nc.tensor.matmul(out=ppb[:], lhsT=SHF[:],
                                         rhs=OB[:, JW - 1, 0:TSEG], start=True,
                                         stop=True)
                        m31 = colp.tile([128, 1], f32, name="m31", tag="m31")
                        mt = colp.tile([128, 1], f32, name="mt", tag="mt")
                        m_own = colp.tile([128, 1], f32, name="m_own", tag="m_own")
                        m_in = colp.tile([128, 1], f32, name="m_in", tag="m_in")
                        lg1 = colp.tile([128, 1], f32, name="lg1", tag="lg1")
                        lg2 = colp.tile([128, 1], f32, name="lg2", tag="lg2")
                        peak = colp.tile([128, 1], f32, name="peak", tag="peak")
                        snew = colp.tile([128, 1], f32, name="snew", tag="snew")
                        dc = colp.tile([128, 1], f32, name="dc", tag="dc")
                        # incoming peak: max over row 31 of prev wave, shifted
                        nc.vector.tensor_reduce(out=m31[:], in_=OL[:, JW - 1, 1:TP2],
                                                axis=Ax.XY, op=Alu.max)
                        nc.vector.tensor_reduce(out=mt[:], in_=OB[:, JW - 1, 0:TP1],
                                                axis=Ax.XY, op=Alu.max)
                        nc.vector.tensor_tensor(out=m31[:], in0=m31[:], in1=mt[:],
                                                op=Alu.max)
                        nc.tensor.matmul(out=pshm[:], lhsT=SHFF[:], rhs=m31[:],
                                         start=True, stop=True)
                        # own peak: final column over all j
                        nc.vector.tensor_reduce(out=m_own[:],
                                                in_=OL[:, :, TP1:TP2],
                                                axis=Ax.XY, op=Alu.max)
                        nc.vector.tensor_reduce(out=lg1[:],
                                                in_=OB[:, :, TSEG:TP1],
                                                axis=Ax.XY, op=Alu.max)
                        nc.vector.tensor_tensor(out=m_own[:], in0=m_own[:],
                                                in1=lg1[:], op=Alu.max)
                        # the TINY floors are load-bearing: m=0 rows (chunks
                        # not yet active) would give rcol/corr = exp(125) =
                        # inf, and 0*inf = NaN poisons later chunks via the
                        # partition shift
                        nc.vector.tensor_scalar(out=m_own[:], in0=m_own[:],
                                                scalar1=TINY, scalar2=1e38,
                                                op0=Alu.max, op1=Alu.min)
                        # clamp fused into the PSUM read; the B0 override is
                        # clamped separately so every row stays >= TINY
                        nc.vector.tensor_scalar(out=m_in[:], in0=pshm[:],
                                                scalar1=TINY, scalar2=1e38,
                                                op0=Alu.max, op1=Alu.min)
                        if qlo == 0:
                            nc.vector.tensor_scalar(out=m_in[0:32, :],
                                                    in0=B0o[:, TSEG:TP1],
                                                    scalar1=TINY, scalar2=1e38,
                                                    op0=Alu.max, op1=Alu.min)
                        # lg = ln(m) via exponent-bits approximation: a single
                        # tensor_copy from the int32 view VALUE-converts to
                        # f32 (ALU ops would treat the bits raw, copies don't)
                        nc.vector.tensor_copy(out=lg1[:],
                                              in_=m_own[:].bitcast(i32))
                        nc.vector.tensor_scalar(out=lg1[:], in0=lg1[:],
                                                scalar1=KLN, scalar2=-FB * KLN,
                                                op0=Alu.mult, op1=Alu.add)
                        nc.vector.tensor_copy(out=lg2[:],
                                              in_=m_in[:].bitcast(i32))
                        nc.vector.tensor_scalar(out=lg2[:], in0=lg2[:],
                                                scalar1=KLN, scalar2=-FB * KLN,
                                                op0=Alu.mult, op1=Alu.add)
                        nc.vector.tensor_tensor(out=lg1[:], in0=lg1[:],
                                                in1=Scol[:], op=Alu.subtract)
                        nc.vector.tensor_tensor(out=lg2[:], in0=lg2[:],
                                                in1=psh[:], op=Alu.subtract)
                        nc.vector.tensor_tensor(out=peak[:], in0=lg1[:],
                                                in1=lg2[:], op=Alu.max)
                        nc.vector.tensor_scalar(out=snew[:], in0=peak[:],
                                                scalar1=-1.0, scalar2=LT,
                                                op0=Alu.mult, op1=Alu.add)
                        # rcol = exp(snew - Scol_old); corr = exp(Scol_new - Sshift)
                        nc.vector.tensor_tensor(out=ARGS[:, 0:1], in0=snew[:],
                                                in1=Scol[:], op=Alu.subtract)
                        for a, b in qsplit(p0, r1):
                            nc.vector.tensor_copy(out=Scol[a:b, :], in_=snew[a:b, :])
                        nc.vector.tensor_tensor(out=ARGS[:, 1:2], in0=Scol[:],
                                                in1=psh[:], op=Alu.subtract)
                        nc.scalar.activation(out=RCE[:], in_=ARGS[:], func=Act.Exp)
                        nc.vector.tensor_copy(out=rcol[:], in_=RCE[:, 0:1])
                        # prev-row handoff at incoming scale correction; the
                        # j=0 stt is fused here: U0 = (d*corr)*ppl + ppb*corr
                        nc.vector.tensor_scalar_mul(PB[:], ppb[:], RCE[:, 1:2])
                        nc.vector.tensor_tensor(out=dc[:], in0=DCOL[:, 0:1],
                                                in1=RCE[:, 1:2], op=Alu.mult)
                        nc.vector.scalar_tensor_tensor(
                            out=UT[:, 0, 1:TP1], in0=ppl[:], scalar=dc[:, 0:1],
                            in1=PB[:], op0=Alu.mult, op1=Alu.add)
                    if w <= NSEG - 1:
                        # chunk-0 row: b0 decay, rescaled (only the final
                        # column of B0c is ever read); u_0 rows 0:32 = b0 row
                        nc.vector.tensor_scalar_mul(B0c[:, TSEG:TP1],
                                                    B0u[:, TSEG:TP1],
                                                    rcol[0:32, :])
                        nc.vector.tensor_scalar_mul(UT[0:32, 0, 1:TP1],
                                                    B0u[:, None, 0:TSEG],
                                                    rcol[0:32, :])
                    # boundary handoff into this wave's tiles (DVE: cheap and
                    # keeps the jloop's first scans same-engine-ordered)
                    nc.vector.tensor_scalar_mul(UT[:, :, 0:1], OL[:, :, TP1:TP2],
                                                rcol[:])
                    nc.vector.tensor_scalar_mul(CL[:, :, 0:1], OB[:, :, TSEG:TP1],
                                                rcol[:])

                with nc.named_scope(f"jloop{w}"):
                    for j in range(JW):
                        if j > 0:  # j=0's stt is fused into wstart
                            nc.vector.scalar_tensor_tensor(
                                out=UT[:, j, 1:TP1], in0=CL[:, j - 1, 1:TP1],
                                scalar=DCOL[:, j:j + 1], in1=CB[:, j - 1, 0:TSEG],
                                op0=Alu.mult, op1=Alu.add)
                        nc.vector.tensor_tensor_scan(
                            out=CL[:, j, 1:TP2], data0=UT[:, j, 0:TP1],
                            data1=QN[P3][:, j, 0:TP1], initial=0.0,
                            op0=Alu.add, op1=Alu.mult)
                        nc.vector.tensor_tensor_scan(
                            out=CB[:, j, 0:TP1], data0=CL[:, j, 0:TP1],
                            data1=QBX[w][:, 0:TP1], initial=0.0,
                            op0=Alu.add, op1=Alu.mult)

                if w >= NSEG - 1:  # chunk q = w - (NSEG-1) finished segment 3
                    q = w - (NSEG - 1)
                    rows = slice(32 * q, 32 * q + 32)
                    nc.vector.tensor_copy(out=STOUT[rows, 0:JW],
                                          in_=CL[rows, :, TP1:TP2])
                    nc.vector.tensor_copy(out=STOUT[rows, JW:2 * JW],
                                          in_=CB[rows, :, TSEG:TP1])

                # next wave's stats/emit/b0 run under this wave's jloop
                late_memsets(w)
                if w + 1 < NW:
                    if w + 1 <= NSEG - 1:
                        stats(w + 1)
                    emit(w + 1)
                    if w + 1 <= NSEG - 1:
                        b0_pre(w + 1)

            nc.vector.tensor_copy(out=STOUT[:, 2 * JW:2 * JW + 1], in_=Scol[:])
            nc.sync.dma_start(out=out_st[:], in_=STOUT[:])
            nc.sync.dma_start(out=out_ll[:], in_=LL[:])
    if split_waits:
        _split_waits(nc)
    return nc


def host_prep(y_true, y_pred):
    """Build per-core input maps."""
    import ml_dtypes

    y_true = np.asarray(y_true).astype(np.int32)
    y_pred = np.asarray(y_pred).astype(np.float32)
    shiftm = np.zeros((128, 256), np.float32)
    for m in range(128):
        shiftm[m - 32 if m >= 32 else m, m] = 1.0          # SHI
        if m >= 32:
            shiftm[m - 32, 128 + m] = 1.0                  # SHF
    in_maps = []
    for core in range(NCORE):
        yt = y_true[core * BLOC:(core + 1) * BLOC]        # [32, 128]
        yp = y_pred[core * BLOC:(core + 1) * BLOC]        # [32, 512, 128]
        ydN = np.ascontiguousarray(yp).reshape(BLOC * NSEG * TSEG, C)
        zbk = np.ascontiguousarray(yp[:, :, BLANK])       # [32, 512]
        # gathered label logits, wave-block layout
        zg = np.take_along_axis(yp, yt.astype(np.int64)[:, None, :], axis=2)
        zgt = zg.transpose(0, 2, 1)                       # [32 b, 128 i, 512 t]
        zlw = np.zeros((512 * JW, TSEG), np.float32)
        base = 0
        for w in range(NW):
            qlo, qhi, p0, p1 = _wave_ranges(w)
            for q in range(qlo, qhi + 1):
                k = w - q
                blk = zgt[:, 32 * q:32 * q + JW, k * TSEG:(k + 1) * TSEG]
                n = BLOC * JW
                zlw[base:base + n] = blk.reshape(n, TSEG)
                base += n
        zlw = zlw.astype(ml_dtypes.bfloat16)
        dcol = np.zeros((128, JW), np.float32)
        for q in range(NCH):
            for j in range(JW):
                i = 32 * q + j + 1
                if i >= 2:
                    dcol[32 * q:32 * q + 32, j] = (
                        yt[:, i - 1] != yt[:, i - 2]).astype(np.float32)
        # segment-0 stats (device computes segments 1-3 under the DP waves)
        yp0 = yp[:, 0:TSEG, :].astype(np.float64)
        den0 = np.exp(yp0).sum(-1)                         # [32, 128]
        pb0 = np.exp(yp0[:, :, BLANK])
        g0 = pb0 + den0 / C
        logg0 = np.log(g0).sum(1)
        logden0 = np.log(den0).sum(1)
        lgc0 = -logg0 / TSEG
        qb0 = np.exp(yp0[:, :, BLANK] + lgc0[:, None]).astype(ml_dtypes.bfloat16)
        st0 = np.stack([lgc0, logg0, logden0], 1).astype(np.float32)
        in_maps.append({"zlw": zlw, "ydN": ydN, "zbk": zbk, "dcol": dcol,
                        "shiftm": shiftm, "qb0": qb0, "st0": st0})
    return in_maps


def host_finish(y_true, results):
    y_true = np.asarray(y_true)
    ll = (y_true != 0).sum(axis=1).astype(np.int64)        # [256]
    losses = np.zeros(B, np.float64)
    for core in range(NCORE):
        res = results[core]
        st = np.asarray(res["out_st"], dtype=np.float64)
        tl, tb, S = st[:, 0:JW], st[:, JW:2 * JW], st[:, 2 * JW:2 * JW + 1]
        ll2 = np.asarray(res["out_ll"])
        logg, logden = ll2[:, 0:NSEG], ll2[:, NSEG:2 * NSEG]
        for b in range(BLOC):
            gb = core * BLOC + b
            l = int(ll[gb])
            q, j = (l - 1) // 32, (l - 1) % 32
            p = 32 * q + b
            logP = np.log(tl[p, j] + tb[p, j])
            losses[gb] = -(logP + logg[b].sum() - logden[b].sum() - S[p, 0])
    return np.float32(losses.mean())


def _get_program():
    if "nc" not in _PROG:
        _PROG["nc"] = build_program()
    return _PROG["nc"]


def kernel(y_true: np.ndarray, y_pred: np.ndarray) -> np.ndarray:
    from concourse.bass_utils import run_bass_kernel_spmd

    nc = _get_program()
    in_maps = host_prep(y_true, y_pred)
    res = run_bass_kernel_spmd(nc, in_maps, core_ids=list(range(NCORE)))
    return host_finish(y_true, res.results)
